# revision 29
# baseline (speedup 1.0000x reference)
"""Trainium2 Bass kernel for nn_MultiLinearCentroids (vq_codebook).

Reference math per class c (C=100, F=128, E=2048, B=512):
  one spectral-norm power-iteration step:
    sigma_c = || W_c (W_c^T u_c) || / || W_c^T u_c ||
  z = x @ W_c^T / sigma_c + b_c                         [B, F]
  probs[:, c] = exp(-||c_c - z||^2 / 2)                 [B]

Sharding: class dim padded 100 -> 104 = 8 cores x 13 classes. x replicated.
Host does only layout transforms (transpose / slice / concat / dtype cast);
all math (including sigma) runs on device.

Key design points (vs. the 127us pipeline that computed t = W^T u as a
GpSimd/DVE elementwise multiply + segmented reduce):
  - sigma via the Gram matrix: G_c = W_c W_c^T is 16 PE matmuls per class
    on the SAME wt chunks the main GEMM uses (lhsT = rhs = wt[:,k,:],
    fp32 PSUM accumulate, ~56ns each, stream-bound).  Then r = G u is
    ONE DVE STT (in0 = G read from PSUM, in1 = u broadcast fp16,
    accum_out fp16 written into the ruc column right before u's column).
  - fused dots+broadcast: one fp16 matmul with lhsT = the r column
    replicated 128x via a stride-0 free dim and rhs = the [r | u] column
    pair gives [rr, ru] = [r.r, u.r] (u.r == ||W^T u||^2) broadcast onto
    all 128 partitions in one shot -- no 1-partition dots, no ScalarE
    copy, no ones-matmul broadcast.  fp16 r costs ~8e-4 probs error
    (the r-rounding mostly cancels between rr and ru) vs the two-pass
    LOW_HIGH fp32 matmul it replaces (~440ns/class).
  - per-class 1/sigma chain on the broadcast pair: exp(0.5 ln(ru/rr)) +
    one Newton step (Ln/Exp/Square all live in the
    natural_log_exp_and_others ACT table set -> single table load).
  - sq = Square(zT * invs + (b - c)) one ScalarE op -> fp16; dist2 =
    ones^T @ sq (fp16 PE partition reduce).  probs = Exp(-0.5 dist2)
    runs one iteration later, emitted AFTER the next Square so a late
    dist2 never blocks the Square/chain path on the ScalarE queue.
  - W, x ship as FP16 (PE 1 cyc/row @2.4GHz, HBM traffic ~9MB); host
    pre-permutes W/x to per-partition-contiguous layouts so each DMA is
    a plain 2D copy.
  - Both the PE (0.65 -> 1.2 -> 2.4 GHz) and the DMA engines
    (~170 -> ~420 GB/s) ramp via a power governor over the first
    ~15us, so: dependency-free warmup matmuls on a memset tile run
    during the DMA prologue to ramp the PE until wt0 lands, and the
    prologue bytes are split across both hardware DGE queues (Sync:
    wt0 + misc/ruc/m16 + u[0:4] + xt6,7; ScalarE: xt0..5 + u[4:]) in
    consumption order.  Classes 0-1 remain DMA-ramp-bound.
  - Pipeline per iteration it on the PE queue: G(it) with dots(it-1)
    embedded after chunk 0, then main GEMM(it) with dist2(it-1)
    embedded after chunk 14; r-STT(it) + chain(it-1) on DVE;
    Square(it-1) + probs(it-2) + chain Ln/Exp on ScalarE.  GpSimd idle.
    Embedding works because the PE ldweights lookahead pipelines
    through same-engine semaphore waits (dots costs 84ns inside the G
    run vs ~420ns at a run boundary); dist2's CROSS-engine wait on
    Square still forces a ~620ns pipeline restart wherever it sits.
    PE is the critical engine at ~4.8us/class steady.  The last class
    hoists its dots ahead of its main GEMM and splits its
    Square/dist2/probs/DMA into half-B pieces so the drain pipelines.
"""

import numpy as np

import concourse.bass as bass
import concourse.tile as tile
from concourse import bacc


class _Bacc(bacc.Bacc):
    """Bacc whose ACT-table pass only sees natural_log_exp_and_others.

    The default pass picks the first table set containing each function
    (natural_log for Ln, exp_and_others for Exp), which alternates sets
    every class = many table loads x ~2.7us. Ln, Exp and Square all live in
    natural_log_exp_and_others, so one load covers the whole kernel."""

    def insert_act_table_loads(self):
        from concourse.hw_specs import get_activation_tables
        has_activation = any(
            isinstance(i, bacc.mybir.InstActivation)
            for b in self.main_func.blocks
            for i in b.instructions
        )
        if not has_activation:
            return
        tables = [(k, v if k == "natural_log_exp_and_others" else type(v)())
                  for k, v in get_activation_tables(self.m.arch).items()]
        bacc._bass_rust.insert_act_table_loads(self, tables)


from concourse import mybir

B = 512
C = 100
E = 2048
F = 128
NCORES = 8
CPAD = 104
CL = CPAD // NCORES  # 13 classes per core
KCH = E // 128       # 16 contraction chunks
KF = KCH * F
NWARM = 9            # p-state warmup matmuls during the DMA prologue

# misc column layout: [b.T | c.T]; (r, u) column pairs live in the
# separate fp16 ruc tile so the dots matmul runs as cheap fp16
MW = 2 * CL

_NC = None


def _emit(tc, d):
    nc = tc.nc
    f32 = mybir.dt.float32
    f16 = mybir.dt.float16
    mult = mybir.AluOpType.mult
    AF = mybir.ActivationFunctionType

    import contextlib
    ctx = contextlib.ExitStack()
    with ctx:
        singles = ctx.enter_context(tc.tile_pool(name="singles", bufs=1))
        wtp = ctx.enter_context(tc.tile_pool(name="wtp", bufs=8))
        sqp = ctx.enter_context(tc.tile_pool(name="sqp", bufs=2))
        smp = ctx.enter_context(tc.tile_pool(name="smp", bufs=4))
        zps = ctx.enter_context(tc.tile_pool(name="zps", bufs=2, space="PSUM"))
        gps = ctx.enter_context(tc.tile_pool(name="gps", bufs=2, space="PSUM"))
        dps = ctx.enter_context(tc.tile_pool(name="dps", bufs=2, space="PSUM"))
        dbp = ctx.enter_context(tc.tile_pool(name="dbp", bufs=2, space="PSUM"))

        # --- PE p-state warmup: dependency-free matmuls on an
        # uninitialized SBUF tile; results land in a dedicated junk PSUM
        # bank that is never read.
        warm_sb = singles.tile([128, B], f16, tag="warm")
        nc.gpsimd.memset(warm_sb, 0.0)
        warm_ps = zps.tile([F, B], f32, tag="zT", name="warm")
        for _ in range(NWARM):
            nc.tensor.matmul(warm_ps, lhsT=warm_sb[:, 0:128], rhs=warm_sb,
                             start=True, stop=True)

        # --- input staging across the two hardware DGE queues (SP +
        # ScalarE): the DMA engines ramp like the PE (~170 GB/s early),
        # so the critical prologue bytes (wt0 + x + u for early classes)
        # are split so both queues land them in parallel.
        ub = d["ubflat"]
        ubc_sb = singles.tile([128, CL * F], f16, tag="ubc")
        misc_sb = singles.tile([128, MW], f32, tag="misc")
        ruc_sb = singles.tile([128, 2 * CL], f16, tag="ruc")
        m16_sb = singles.tile([128, 1], f16, tag="m16")
        ones_sb = m16_sb[:, 0:1]

        def ubc_dma(eng, c0, c1):
            ub_sl = ub[c0 * F:c1 * F]
            ub_b = bass.AP(tensor=ub_sl.tensor, offset=ub_sl.offset,
                           ap=[[0, 128]] + [list(a) for a in ub_sl.ap])
            eng.dma_start(out=ubc_sb[:, c0 * F:c1 * F], in_=ub_b)

        # W trigger groups: two singles first (fast pipeline start), then
        # pairs; issued interleaved with half-size x groups so class 0's
        # main GEMM is never waiting on x.
        WGROUPS = [[0], [1], [2, 3], [4, 5], [6, 7], [8, 9], [10, 11], [12]]
        wt_of = {}

        def wt_dma(gi):
            cls = WGROUPS[gi]
            wt = wtp.tile([128, len(cls), KCH, F], f16, tag="wt",
                          name=f"wtg{gi}")
            nc.sync.dma_start(
                out=wt, in_=d["wt"][:, cls[0] * KF:(cls[-1] + 1) * KF
                                    ].rearrange("p (c k f) -> p c k f",
                                                k=KCH, f=F))
            for i, c in enumerate(cls):
                wt_of[c] = (wt, i)

        def wtc(c):
            t, i = wt_of[c]
            return t[:, i, :, :]

        # Sync queue: wt0 (class 0's G gates on it), the small misc/m16/
        # early-u tiles, then the last two x groups (consumed last by
        # class 0's GEMM) and the remaining W groups which trail the
        # compute easily.  ScalarE queue in parallel: the first six x
        # groups in consumption order, then the remaining u classes.
        XG = 2  # x chunks per staging DMA
        NXG = KCH // XG
        xt_tiles = [None] * NXG
        wt_dma(0)

        def xt_dma(g, eng):
            xg = singles.tile([128, XG, B], f16, tag=f"xt{g}",
                              name=f"xt{g}")
            eng.dma_start(
                out=xg, in_=d["xt"][:, g * XG * B:(g + 1) * XG * B
                                    ].rearrange("p (k b) -> p k b", b=B))
            xt_tiles[g] = xg

        for g in range(NXG - 2):
            xt_dma(g, nc.scalar)
        nc.sync.dma_start(out=ruc_sb, in_=d["ruc16"][:, 0:2 * CL])
        nc.sync.dma_start(out=m16_sb, in_=d["m16"][:, 0:1])
        ubc_dma(nc.sync, 0, 1)
        for g in range(NXG - 2, NXG):
            xt_dma(g, nc.sync)
        nc.sync.dma_start(out=misc_sb, in_=d["misc"][:, 0:MW])
        ubc_dma(nc.sync, 1, 4)
        ubc_dma(nc.scalar, 4, CL)
        for gi in range(1, len(WGROUPS)):
            wt_dma(gi)

        negm_sb = singles.tile([F, CL], f32, tag="negm")
        nc.vector.tensor_sub(negm_sb, misc_sb[:, :CL], misc_sb[:, CL:2 * CL])

        def xchunk(k):
            return xt_tiles[k // XG][:, k % XG, :]

        st = [dict() for _ in range(CL)]

        def emit_rstt(cr):
            """r = G u: one DVE STT accumulating into the ruc column."""
            s = st[cr]
            scr = smp.tile([128, F], f32, tag="scr")
            with nc.allow_low_precision(reason="r feeds fp16 dots matmul"):
                nc.vector.scalar_tensor_tensor(
                    out=scr, in0=s["G"], scalar=1.0,
                    in1=ubc_sb[:, cr * F:(cr + 1) * F],
                    op0=mult, op1=mult,
                    accum_out=ruc_sb[:, 2 * cr:2 * cr + 1])

        def emit_dots(cr, skip_check=False):
            """fused dots+broadcast: lhsT = r replicated 128x (stride-0
            free dim), rhs = [r | u] -> out[m, :] = [r.r, u.r] for all m."""
            s = st[cr]
            rcol = ruc_sb[:, 2 * cr:2 * cr + 1]
            db = dbp.tile([128, 2], f32, tag="db")
            s["db"] = db
            rrep = bass.AP(tensor=rcol.tensor, offset=rcol.offset,
                           ap=[list(rcol.ap[0]), [0, 128]])
            nc.tensor.matmul(
                db, lhsT=rrep,
                rhs=ruc_sb[:, 2 * cr:2 * cr + 2],
                start=True, stop=True, skip_group_check=skip_check)

        def emit_sigma(cr):
            emit_rstt(cr)
            emit_dots(cr)

        def emit_chain(cr):
            s = st[cr]
            db = s["db"]
            recip = smp.tile([128, 1], f32, tag="recip")
            nc.vector.reciprocal(recip, db[:, 0:1])
            invs2 = smp.tile([128, 1], f32, tag="invs2")
            nc.vector.tensor_mul(invs2, recip, db[:, 1:2])
            lnr = smp.tile([128, 1], f32, tag="lnr")
            nc.scalar.activation(out=lnr, in_=invs2, func=AF.Ln)
            invs0 = smp.tile([128, 1], f32, tag="invs0")
            nc.scalar.activation(out=invs0, in_=lnr, func=AF.Exp, scale=0.5)
            # one Newton step y1 = (y0 + a/y0)/2 tightens the LUT
            # exp(0.5 ln a) sqrt estimate to ~1 ulp; probs error is
            # a large multiple of the relative sigma error.
            ry = smp.tile([128, 1], f32, tag="ry")
            nc.vector.reciprocal(ry, invs0)
            ar = smp.tile([128, 1], f32, tag="ar")
            nc.vector.tensor_mul(ar, invs2, ry)
            hsum = smp.tile([128, 1], f32, tag="hsum")
            nc.vector.tensor_add(hsum, invs0, ar)
            invs = smp.tile([128, 1], f32, tag="invs")
            nc.vector.tensor_scalar_mul(invs, hsum, 0.5)
            s["invs"] = invs

        def emit_dist2(cd, skip_check=False):
            s = st[cd]
            d2 = dps.tile([1, B], f32, tag="d2")
            s["d2"] = d2
            if cd == CL - 1:
                for h in (0, 1):
                    hs_ = slice(h * (B // 2), (h + 1) * (B // 2))
                    nc.tensor.matmul(d2[:, hs_], lhsT=ones_sb,
                                     rhs=s["sq"][:, hs_],
                                     start=True, stop=True,
                                     skip_group_check=skip_check)
            else:
                nc.tensor.matmul(d2, lhsT=ones_sb, rhs=s["sq"],
                                 start=True, stop=True,
                                 skip_group_check=skip_check)

        def emit_square(cd):
            s = st[cd]
            sq = sqp.tile([F, B], f16, tag="sq")
            s["sq"] = sq
            if cd == CL - 1:
                # drain: halve so dist2/probs pipeline behind Square
                for h in (0, 1):
                    nc.scalar.activation(
                        out=sq[:, h * (B // 2):(h + 1) * (B // 2)],
                        in_=s["zT"][:, h * (B // 2):(h + 1) * (B // 2)],
                        func=AF.Square,
                        bias=negm_sb[:, cd:cd + 1], scale=s["invs"])
            else:
                nc.scalar.activation(
                    out=sq, in_=s["zT"], func=AF.Square,
                    bias=negm_sb[:, cd:cd + 1], scale=s["invs"])

        def emit_probs(cd2):
            s = st[cd2]
            probs_c = smp.tile([1, B], f32, tag="probs_c")
            if cd2 == CL - 1:
                for h in (0, 1):
                    hs_ = slice(h * (B // 2), (h + 1) * (B // 2))
                    nc.scalar.activation(
                        out=probs_c[:, hs_], in_=s["d2"][:, hs_],
                        func=AF.Exp, scale=-0.5)
                    nc.sync.dma_start(
                        out=d["out"][0:1, cd2 * B + h * (B // 2):
                                     cd2 * B + (h + 1) * (B // 2)],
                        in_=probs_c[:, hs_])
            else:
                nc.scalar.activation(
                    out=probs_c, in_=s["d2"], func=AF.Exp, scale=-0.5)
                nc.sync.dma_start(out=d["out"][0:1, cd2 * B:(cd2 + 1) * B],
                                  in_=probs_c)

        for it in range(CL + 2):
            cb, cd, cd2 = it, it - 1, it - 2
            last = cb == CL - 1

            if cb < CL:
                # ---- PE: G(cb) with dots(cb-1) embedded after chunk 0
                # (the ldweights prefetch pipeline hides it there, vs
                # ~310ns at a run boundary)
                s = st[cb]
                wt = wtc(cb)
                G = gps.tile([128, F], f32, tag="G")
                s["G"] = G
                for k in range(KCH):
                    nc.tensor.matmul(
                        G, lhsT=wt[:, k, :], rhs=wt[:, k, :],
                        start=(k == 0), stop=(k == KCH - 1),
                        skip_group_check=True)
                    if k == 0 and 0 <= cb - 1:
                        emit_dots(cb - 1, skip_check=True)
                # ---- chain(cb-1): its dots just ran; then Square(cd)
                # (same class, invs from this chain) and probs(cd2)
                if 0 <= cb - 1:
                    emit_chain(cb - 1)
                if 0 <= cd:
                    emit_square(cd)
                if 0 <= cd2:
                    emit_probs(cd2)
                # last class: sigma path ahead of the main GEMM so its
                # chain finishes during the GEMM instead of in the drain
                if last:
                    emit_sigma(cb)
                    emit_chain(cb)
                # ---- PE: main GEMM(cb) with dist2(cd) embedded late
                # enough that Square(cd) has finished
                zT = zps.tile([F, B], f32, tag="zT")
                s["zT"] = zT
                for k in range(KCH):
                    nc.tensor.matmul(
                        zT, lhsT=wt[:, k, :], rhs=xchunk(k),
                        start=(k == 0), stop=(k == KCH - 1),
                        skip_group_check=True)
                    if k == KCH - 2 and 0 <= cd:
                        emit_dist2(cd, skip_check=True)
                if not last:
                    emit_rstt(cb)
            else:
                # ---- drain iterations: no PE runs left to embed into
                if 0 <= cd < CL:
                    emit_square(cd)
                if 0 <= cd2 < CL:
                    emit_probs(cd2)
                if 0 <= cd < CL:
                    emit_dist2(cd)


def _build():
    nc = _Bacc(trn_type="TRN2", target_bir_lowering=False, debug=False,
               num_devices=NCORES)
    f32 = mybir.dt.float32
    f16 = mybir.dt.float16
    d = {
        "wt": nc.dram_tensor("wt", [128, CL * KCH * F], f16,
                             kind="ExternalInput").ap(),
        "xt": nc.dram_tensor("xt", [128, KCH * B], f16,
                             kind="ExternalInput").ap(),
        "misc": nc.dram_tensor("misc", [128, MW], f32,
                               kind="ExternalInput").ap(),
        "ruc16": nc.dram_tensor("ruc16", [128, 2 * CL], f16,
                                kind="ExternalInput").ap(),
        "m16": nc.dram_tensor("m16", [128, 1], f16,
                              kind="ExternalInput").ap(),
        "ubflat": nc.dram_tensor("ubflat", [CL * F], f16,
                                 kind="ExternalInput").ap(),
        "out": nc.dram_tensor("out", [1, CL * B], f32,
                              kind="ExternalOutput").ap(),
    }
    with tile.TileContext(nc) as tc:
        _emit(tc, d)
    nc.compile()
    return nc


def _get_nc():
    global _NC
    if _NC is None:
        _NC = _build()
    return _NC


def make_in_maps(inputs):
    x = np.ascontiguousarray(inputs["x"], dtype=np.float32)
    W = np.ascontiguousarray(inputs["W"], dtype=np.float32)
    b = np.ascontiguousarray(inputs["b"], dtype=np.float32)
    u = np.ascontiguousarray(inputs["u"], dtype=np.float32)
    c = np.ascontiguousarray(inputs["c"], dtype=np.float32)
    pad = CPAD - C
    Wp = np.concatenate([W, W[:pad]], axis=0)
    bp = np.concatenate([b, b[:pad]], axis=0)
    up = np.concatenate([u, u[:pad]], axis=0)
    cp = np.concatenate([c, c[:pad]], axis=0)
    # pre-permute to per-partition-contiguous fp16 layouts so device DMAs
    # are simple 2D copies (cheap SP triggers, full-row HBM reads):
    # wt[p, c, k, f] = W[c, f, 128k+p];  xt[p, k, b] = x[b, 128k+p]
    WT = Wp.transpose(0, 2, 1).reshape(CPAD, KCH, 128, F)
    xt = np.ascontiguousarray(x.T.reshape(KCH, 128, B).transpose(1, 0, 2)
                              .reshape(128, KCH * B).astype(np.float16))
    m16 = np.ones((128, 1), dtype=np.float16)
    in_maps = []
    for ci in range(NCORES):
        sl = slice(ci * CL, (ci + 1) * CL)
        ruc = np.zeros((128, 2 * CL), dtype=np.float16)
        ruc[:, 1::2] = up[sl].T.astype(np.float16)
        in_maps.append({
            "wt": np.ascontiguousarray(
                WT[sl].transpose(2, 0, 1, 3).reshape(128, CL * KCH * F)
                .astype(np.float16)),
            "xt": xt,
            "ubflat": np.ascontiguousarray(
                up[sl].reshape(-1).astype(np.float16)),
            "misc": np.ascontiguousarray(np.concatenate(
                [bp[sl].T, cp[sl].T], axis=1)),
            "ruc16": np.ascontiguousarray(ruc),
            "m16": m16,
        })
    return in_maps


def run_spmd(in_maps, **kw):
    from concourse.bass_utils import run_bass_kernel_spmd
    return run_bass_kernel_spmd(_get_nc(), in_maps, list(range(NCORES)), **kw)


def gather_output(results):
    rows = np.concatenate(
        [results[i]["out"].reshape(CL, B) for i in range(NCORES)], axis=0)
    return np.ascontiguousarray(rows[:C].T)  # [B, C] float32


def kernel(**inputs):
    bkr = run_spmd(make_in_maps(inputs))
    return gather_output(bkr.results)
